# revision 8
# baseline (speedup 1.0000x reference)
"""Sparse (top-64) attention kernel for Trainium2, 8 NeuronCores.

Problem: B=32, LQ=LK=2048, D=DV=64, TOPK=64, fp32.
  dots = Q @ K^T            [B, Lq, Lk]
  top64 selection per (b, q) row, softmax(top_dots * D**-0.5), gather V, contract.

Sharding: batch dim B across 8 cores (4 batches/core), full K/V per batch local.

Per-core algorithm (per batch, per 128-query tile):
  1. PE: S = Q_tile @ K^T -> PSUM [128, 2048] fp32 (fp32 matmul: selection-set
     fidelity vs the fp32 reference requires full-precision scores).
  2. ACT: E = exp(S * scale): PSUM -> SBUF fp32 (monotone; selection on E).
  3. DVE: exact top-64 threshold t via per-128-chunk top-16 candidates
     (max8 + match_replace + max8 -> 256 candidates/row; top-64 of candidates
     via 8 rounds of max8/match_replace; t = 64th largest).
  4. t' = t*(1-2^-23): strictly between the 64th and 65th values, so
     sign(E - t') is exactly +/-1 (never 0) and relu(E - t') > 0 iff selected.
  5. ACT: R = relu(E - t') bf16, G = sign(E - t') bf16 (one pass each).
     Identity: sum_sel e_i v_i = sum R_i v_i + t' * sum m_i v_i with
     m = (G+1)/2, and Z = sum R + t' * count. All selection-exact.
  6. DMA xbar transpose (off-engine): R^T, G^T [128, 16, 128] bf16.
  7. PE: Racc = sum_c R^T_c.T @ [V_c|1]; A = sum_c (G^T_c.T + ones.T) @ [V_c|1]
     (the ones-matmuls fold in column sums so A = sum G v + sum v = 2*sum m v).
  8. DVE: NUM = (t'/2)*A + Racc  -> [.., 0:64] = numerator, [.., 64] = Z;
     out = NUM[:, :64] * (1/Z); DMA to output.
"""

import numpy as np

B, LQ, LK, D, DV, TOPK = 32, 2048, 2048, 64, 64, 64
N_CORES = 8
B_PER_CORE = B // N_CORES
SCALE = float(D) ** -0.5
ONE_MINUS_EPS = float(np.float32(1.0) - np.float32(2.0 ** -23))

_CACHE = {}


def _patch_tile_drain():
    """walrus codegen rejects >2 sem-waits on one CTRL; split the tail-drain
    waits across single-wait NOPs."""
    import concourse.mybir as mybir
    from concourse.tile import TileContext, ScopedClock

    if getattr(TileContext, "_drain_patched", False):
        return

    def _drain_and_barrier(self, tick_clock, wait_clock):
        nc = self.nc
        probe = nc.sync.nop(nofuse=True)
        wait_clock.add_sem_waits(probe.ins, ScopedClock({None: tick_clock.global_clock}))
        si = probe.ins.sync_info
        waits = list(si.on_wait) if si is not None else []
        if len(waits) > 1:
            probe.ins.sync_info = mybir.SyncInfo(
                on_wait=waits[:1], on_update=list(si.on_update)
            )
            rest = waits[1:]
            while rest:
                n2 = nc.sync.nop(nofuse=True)
                n2.ins.sync_info = mybir.SyncInfo(on_wait=rest[:1], on_update=[])
                rest = rest[1:]
        nc.sync.drain()
        nc.all_engine_barrier()
        assert self.sems is not None
        popped = nc._tile_sem_poison_stack.pop()
        assert popped is self._sem_poison
        nc.clear_and_free_semaphores(list(self.sems.allocated().values()))
        nc.all_engine_barrier()

    TileContext._drain_and_barrier = _drain_and_barrier
    TileContext._drain_patched = True


def _split_sync_waits(nc):
    """This walrus build accepts at most ONE sem-wait per instruction; hoist
    excess waits onto single-wait NOPs inserted just before, same engine."""
    import concourse.mybir as mybir

    n_new = 0
    for f in nc.m.functions:
        for bb in f.blocks:
            out = []
            changed = False
            for inst in bb.instructions:
                si = inst.sync_info
                waits = list(si.on_wait) if si is not None else []
                if len(waits) > 1:
                    changed = True
                    for w in waits[:-1]:
                        nop = mybir.InstNoOp(
                            name=f"WSPLIT-{n_new}", ins=[], outs=[]
                        )
                        n_new += 1
                        nop.engine = inst.engine
                        nop.sync_info = mybir.SyncInfo(on_wait=[w], on_update=[])
                        out.append(nop)
                    inst.sync_info = mybir.SyncInfo(
                        on_wait=[waits[-1]], on_update=list(si.on_update)
                    )
                out.append(inst)
            if changed:
                bb.instructions = out


def build(n_batches=B_PER_CORE, n_qtiles=LQ // 128):
    import concourse.bass as bass
    import concourse.tile as tile
    from concourse import mybir

    _patch_tile_drain()

    F32 = mybir.dt.float32
    BF16 = mybir.dt.bfloat16
    I32 = mybir.dt.int32
    AOP = mybir.AluOpType
    AF = mybir.ActivationFunctionType

    nc = bass.Bass(trn_type="TRN2")
    q_d = nc.dram_tensor("Q", [n_batches, LQ, D], F32, kind="ExternalInput")
    k_d = nc.dram_tensor("K", [n_batches, LK, D], F32, kind="ExternalInput")
    v_d = nc.dram_tensor("V", [n_batches, LK, DV], F32, kind="ExternalInput")
    o_d = nc.dram_tensor("O", [n_batches, LQ, DV], F32, kind="ExternalOutput")
    ident_d = nc.inline_tensor(np.eye(128, dtype=np.float32), name="ident")

    NKC = LK // 128  # 16 k-chunks
    DV1 = DV + 1     # V plus ones column

    from contextlib import ExitStack

    with tile.TileContext(nc) as tc, ExitStack() as ctx:
        consts = ctx.enter_context(tc.tile_pool(name="consts", bufs=1))
        batchp = ctx.enter_context(tc.tile_pool(name="batchp", bufs=2))
        work = ctx.enter_context(tc.tile_pool(name="work", bufs=2))
        small = ctx.enter_context(tc.tile_pool(name="small", bufs=4))
        ps_s = ctx.enter_context(tc.tile_pool(name="ps_s", bufs=1, space="PSUM"))
        ps_t = ctx.enter_context(tc.tile_pool(name="ps_t", bufs=2, space="PSUM"))
        ps_o = ctx.enter_context(tc.tile_pool(name="ps_o", bufs=1, space="PSUM"))

        ident = consts.tile([128, 128], F32)
        nc.sync.dma_start(out=ident, in_=ident_d[:])
        allones = consts.tile([128, 128], BF16)
        nc.vector.memset(allones, 1.0)

        for b in range(n_batches):
            # ---- batch prologue: QT/KT (d-major fp32) + V chunks bf16 ----
            qt = batchp.tile([64, LQ], F32, tag="qt")
            kt = batchp.tile([64, LK], F32, tag="kt")
            vsb = batchp.tile([128, NKC, DV1], BF16, tag="vsb")
            vld = batchp.tile([128, NKC, DV], F32, tag="vld")
            nc.sync.dma_start(
                out=vld, in_=v_d[b].rearrange("(c p) d -> p c d", p=128)
            )
            # cast V to bf16 (DVE 2x_2p) + ones column
            nc.vector.tensor_copy(vsb[:, :, 0:DV], vld)
            nc.vector.memset(vsb[:, :, DV:DV1], 1.0)
            for dst, src in ((qt, q_d), (kt, k_d)):
                ldall = batchp.tile([128, NKC * D], F32, tag="ldall")
                nc.sync.dma_start(
                    out=ldall,
                    in_=src[b].rearrange("(c p) d -> p c d", p=128),
                )
                for s in range(4):  # slabs of 4 tiles = 512 columns
                    slab = ps_t.tile([128, 512], F32, tag="pt")
                    for u in range(4):
                        t_i = 4 * s + u
                        nc.tensor.transpose(
                            out=slab[:64, u * 128 : (u + 1) * 128],
                            in_=ldall[:, t_i * D : (t_i + 1) * D],
                            identity=ident,
                        )
                    nc.scalar.activation(
                        out=dst[:, s * 512 : (s + 1) * 512],
                        in_=slab[:64, :],
                        func=AF.Copy,
                    )

            for i in range(n_qtiles):
                # ---- 1. S = Q_tile @ K^T (fp32) ----
                s_ps = ps_s.tile([128, LK], F32, tag="s")
                for j in range(LK // 512):
                    nc.tensor.matmul(
                        out=s_ps[:, j * 512 : (j + 1) * 512],
                        lhsT=qt[:, i * 128 : (i + 1) * 128],
                        rhs=kt[:, j * 512 : (j + 1) * 512],
                        start=True,
                        stop=True,
                    )
                # ---- 2. E = exp(S * scale) ----
                e = work.tile([128, LK], F32, tag="e")
                nc.scalar.activation(out=e, in_=s_ps, func=AF.Exp, scale=SCALE)
                # ---- 3. exact top-64 threshold ----
                cand = work.tile([128, 256], F32, tag="cand")
                for c in range(NKC):
                    ech = e[:, c * 128 : (c + 1) * 128]
                    nc.vector.max(out=cand[:, c * 16 : c * 16 + 8], in_=ech)
                    ez = small.tile([128, 128], F32, tag="ez")
                    nc.vector.match_replace(
                        out=ez,
                        in_to_replace=cand[:, c * 16 : c * 16 + 8],
                        in_values=ech,
                        imm_value=0.0,
                    )
                    nc.vector.max(out=cand[:, c * 16 + 8 : c * 16 + 16], in_=ez)
                m8 = None
                for r in range(8):
                    m8 = small.tile([128, 8], F32, tag="m8")
                    if r == 0:
                        # chunk ranks 9-16 are each dominated by their own
                        # chunk's top-8, so the global top-8 lies within the
                        # 128 first-round candidates — scan only those.
                        nc.vector.max(
                            out=m8,
                            in_=cand.rearrange("p (c t) -> p c t", t=16)[:, :, :8],
                        )
                    else:
                        nc.vector.max(out=m8, in_=cand)
                    if r < 7:
                        if r == 0:
                            c1 = cand.rearrange("p (c t) -> p c t", t=16)[:, :, :8]
                            nc.vector.match_replace(
                                out=c1, in_to_replace=m8, in_values=c1, imm_value=0.0
                            )
                        else:
                            nc.vector.match_replace(
                                out=cand, in_to_replace=m8, in_values=cand, imm_value=0.0
                            )
                thr = m8[:, 7:8]
                # ---- 4. t' strictly inside (t65, t64); -t' and t'/2 (ACT) ----
                tp = small.tile([128, 1], F32, tag="tp")
                nc.scalar.activation(out=tp, in_=thr, func=AF.Copy, scale=ONE_MINUS_EPS)
                tn = small.tile([128, 1], F32, tag="tn")
                nc.scalar.activation(out=tn, in_=tp, func=AF.Copy, scale=-1.0)
                th = small.tile([128, 1], F32, tag="th")
                nc.scalar.activation(out=th, in_=tp, func=AF.Copy, scale=0.5)
                # ---- 5. R = relu(E - t') bf16, G = sign(E - t') bf16 ----
                r16 = work.tile([128, LK], BF16, tag="r16")
                nc.scalar.activation(out=r16, in_=e, func=AF.Relu, bias=tn, scale=1.0)
                g16 = work.tile([128, LK], BF16, tag="g16")
                nc.scalar.activation(out=g16, in_=e, func=AF.Sign, bias=tn, scale=1.0)
                # ---- 6. off-engine transposes via DMA xbar ----
                rt = work.tile([128, NKC, 128], BF16, tag="rt")
                nc.sync.dma_start_transpose(rt, r16)
                gt = work.tile([128, NKC, 128], BF16, tag="gt")
                nc.sync.dma_start_transpose(gt, g16)
                # ---- 7. PV matmuls ----
                racc = ps_o.tile([128, DV1], F32, tag="racc")
                for c in range(NKC):
                    nc.tensor.matmul(
                        out=racc,
                        lhsT=rt[:, c, :],
                        rhs=vsb[:, c, :],
                        start=(c == 0),
                        stop=(c == NKC - 1),
                    )
                gacc = ps_o.tile([128, DV1], F32, tag="gacc")
                for c in range(NKC):
                    nc.tensor.matmul(
                        out=gacc,
                        lhsT=gt[:, c, :],
                        rhs=vsb[:, c, :],
                        start=(c == 0),
                        stop=False,
                    )
                for c in range(NKC):
                    nc.tensor.matmul(
                        out=gacc,
                        lhsT=allones,
                        rhs=vsb[:, c, :],
                        start=False,
                        stop=(c == NKC - 1),
                    )
                # ---- 8. combine + normalize ----
                rsb = small.tile([128, DV1], F32, tag="rsb")
                nc.scalar.activation(out=rsb, in_=racc, func=AF.Copy)
                num = small.tile([128, DV1], F32, tag="num")
                nc.vector.scalar_tensor_tensor(
                    out=num,
                    in0=gacc,
                    scalar=th,
                    in1=rsb,
                    op0=AOP.mult,
                    op1=AOP.add,
                )
                rz = small.tile([128, 1], F32, tag="rz")
                nc.vector.reciprocal(out=rz, in_=num[:, DV:DV1])
                osb = small.tile([128, DV], F32, tag="osb")
                nc.vector.tensor_scalar(osb, num[:, 0:DV], rz, None, AOP.mult)
                nc.sync.dma_start(
                    out=o_d[b, i * 128 : (i + 1) * 128, :], in_=osb
                )

    _split_sync_waits(nc)
    return nc


def _get_nc(key, **kw):
    if key not in _CACHE:
        _CACHE[key] = build(**kw)
    return _CACHE[key]


def kernel(Q, K, V, topk):
    assert int(topk) == TOPK
    Q = np.ascontiguousarray(np.asarray(Q, dtype=np.float32))
    K = np.ascontiguousarray(np.asarray(K, dtype=np.float32))
    V = np.ascontiguousarray(np.asarray(V, dtype=np.float32))

    from concourse.bass_utils import run_bass_kernel_spmd

    nc = _get_nc("full")
    in_maps = []
    for c in range(N_CORES):
        sl = slice(c * B_PER_CORE, (c + 1) * B_PER_CORE)
        in_maps.append(
            {
                "Q": np.ascontiguousarray(Q[sl]),
                "K": np.ascontiguousarray(K[sl]),
                "V": np.ascontiguousarray(V[sl]),
            }
        )
    res = run_bass_kernel_spmd(nc, in_maps, core_ids=list(range(N_CORES)))
    global LAST_EXEC_NS
    LAST_EXEC_NS = res.exec_time_ns
    out = np.concatenate([res.results[c]["O"] for c in range(N_CORES)], axis=0)
    return out.astype(np.float32)


LAST_EXEC_NS = None
